# revision 28
# baseline (speedup 1.0000x reference)
"""Causal multi-head self-attention (B=4, S=2048, D=1024, H=16) on 8 TRN2
NeuronCores.

Sharding: core c = (batch b = c//2, head-half = c%2). Each core computes, for
its batch and its 8 heads: fused QKV projections (+RoPE), causal softmax
attention, and a row-sharded output projection; the host sums the two partial
y's per batch.

v2 (vs the fp32r baseline): all matmul operands are bf16 (halves DMA, removes
the fp32r narrow-moving 4x penalty, enables DVE 2x modes); q/k/v projections
share one pass over x (no phase boundary); RoPE's partition swap runs as 4
small SBUF->SBUF DMAs off the compute engines (ScalarE downcasts the PSUM
projection to bf16, DVE does 2 muls + 1 add at 2x) instead of 4 full-price
partition-sliced DVE muls; attention is query-block-outer so the output
projection of block j interleaves into block j+1's PE stream; per-head [65,512]
PSUM accumulators (+ ones column emitting softmax denominators) keep all 8
PSUM banks allocated: 2x2 scores + 3 pa + 1 output-projection.

Device layouts (per core):
  qT,kT: [128, 4, 2048] bf16; chunk hc = heads (2hc, 2hc+1); within a head's 64
         rows: [even dims (32) | odd dims (32)].
  v:     [128, 16, 8, 65] bf16 = [t % 128, t//128, head, dim+ones]; the ones
         column makes the AV matmul emit the softmax denominator as row 64.
  scores are built transposed (S.T[t, s]) so exp(S.T) feeds the AV matmul as
  the moving operand with no transposes anywhere.
"""

import numpy as np

B, S, D = 4, 2048, 1024
NUM_HEADS = 16
THETA = 10000.0
DH = 64
N_CORES = 8
P = 128

_CACHE = {}


def build_nc():
    """Build the single-core SPMD Bass program (identical on all 8 cores)."""
    import concourse.mybir as mybir
    import concourse.tile as tile
    from concourse import bacc
    from concourse.bass import ts

    F32 = mybir.dt.float32
    BF16 = mybir.dt.bfloat16
    Act = mybir.ActivationFunctionType

    nc = bacc.Bacc(trn_type="TRN2")
    xT_d = nc.dram_tensor("xT", [D, S], BF16, kind="ExternalInput")
    wqT_d = nc.dram_tensor("wqT", [D, 512], BF16, kind="ExternalInput")
    wkT_d = nc.dram_tensor("wkT", [D, 512], BF16, kind="ExternalInput")
    wvT_d = nc.dram_tensor("wvT", [D, 512], BF16, kind="ExternalInput")
    woT_d = nc.dram_tensor("woT", [512, D], BF16, kind="ExternalInput")
    cosT_d = nc.dram_tensor("cosT", [P, S], BF16, kind="ExternalInput")
    sinT_d = nc.dram_tensor("sinT", [P, S], BF16, kind="ExternalInput")
    tri_d = nc.dram_tensor("tri", [P, P], BF16, kind="ExternalInput")
    y_d = nc.dram_tensor("y", [S, D], F32, kind="ExternalOutput")

    xT3 = xT_d.ap().rearrange("(kc p) s -> p kc s", p=P)     # [128, 8, 2048]
    wq3 = wqT_d.ap().rearrange("(kc p) j -> p kc j", p=P)    # [128, 8, 512]
    wk3 = wkT_d.ap().rearrange("(kc p) j -> p kc j", p=P)
    wv3 = wvT_d.ap().rearrange("(kc p) j -> p kc j", p=P)
    wo3 = woT_d.ap().rearrange("(jc p) i -> p jc i", p=P)    # [128, 4, 1024]
    y_ap = y_d.ap()

    with tile.TileContext(nc) as tc:
        with tc.tile_pool(name="pers", bufs=1) as pers:
            qT = pers.tile([P, 4, S], BF16)
            kT = pers.tile([P, 4, S], BF16)
            vA = pers.tile([P, 16, 8, 65], BF16)
            outT = pers.tile([P, 4, S], BF16)
            wo_s = pers.tile([P, 4, D], BF16)
            trib = pers.tile([P, P], BF16)
            cosb = pers.tile([P, S], BF16)
            sinb = pers.tile([P, S], BF16)

            # ---- Fully fused: q/k/v projections + RoPE, attention blocks
            # ---- interleaved as their key slices complete, and the output
            # ---- projection of each finished block under the next one.
            # PSUM: psA (proj + p3, 2x1 bank) + psB (scores, 2x2) + psC
            # (per-head AV accumulators, 2x1) = 8 banks.
            with (
                tc.tile_pool(name="w1", bufs=1) as w1,
                tc.tile_pool(name="x1", bufs=2) as x1,
                tc.tile_pool(name="rt", bufs=3) as rt,
                tc.tile_pool(name="ptp", bufs=4) as ptp,
                tc.tile_pool(name="rcp", bufs=4) as rcp,
                tc.tile_pool(name="rbp", bufs=4) as rbp,
                tc.tile_pool(name="ysb", bufs=3) as ysb,
                tc.tile_pool(name="drm", bufs=2, space="DRAM") as drm,
                tc.tile_pool(name="psA", bufs=2, space="PSUM") as psA,
                tc.tile_pool(name="psB", bufs=2, space="PSUM") as psB,
                tc.tile_pool(name="psC", bufs=2, space="PSUM") as psC,
            ):
                wq_s = w1.tile([P, 8, 512], BF16)
                wk_s = w1.tile([P, 8, 512], BF16)
                wv_s = w1.tile([P, 8, 512], BF16)
                ones8 = w1.tile([P, 8], F32)

                # DMA priority order: first q-projection inputs, then rope
                # tables + causal mask (attention(0) starts early), then k/v
                # weights, then the output-projection constants.
                xs0 = x1.tile([P, 8, 512], BF16, tag="xs")
                for kc in range(8):
                    nc.sync.dma_start(xs0[:, kc, :], xT3[:, kc, 0:512])
                    nc.sync.dma_start(wq_s[:, kc, :], wq3[:, kc, :])
                nc.sync.dma_start(cosb[:], cosT_d.ap())
                nc.sync.dma_start(sinb[:], sinT_d.ap())
                nc.sync.dma_start(trib[:], tri_d.ap())
                for kc in range(8):
                    nc.sync.dma_start(wk_s[:, kc, :], wk3[:, kc, :])
                nc.sync.dma_start(wv_s[:], wv3)
                nc.sync.dma_start(wo_s[:], wo3)

                nc.vector.memset(ones8[:], 1.0)
                for t16 in range(16):
                    nc.vector.tensor_copy(vA[:, t16, :, 64:65], ones8.unsqueeze(2))

                # RoPE strategy: partial-partition (channels<128) engine ops
                # cost ~3x on HW, so every DVE op here is full-channel. Per
                # (sl, matrix): ScalarE stages the 4 projection chunks into
                # pall (bf16); the 32-row partition swap runs as 4 batched
                # SBUF->SBUF DMAs over the whole staging tile; DVE then does
                # 3 full-width ops per chunk: dst = pall*cos; tB = psw*sinS
                # (sign folded per destination row); dst += tB.
                def rope_finish(pall, dstT, sls):
                    # sin-side on GpSimd (SBUF-only ops): DVE's FIFO must keep
                    # draining the projection PSUM ring
                    psw = rt.tile([P, 4, 512], BF16, tag="psw")
                    for hb in (0, 64):
                        nc.sync.dma_start(
                            psw[hb : hb + 32, :, :], pall[hb + 32 : hb + 64, :, :]
                        )
                        nc.sync.dma_start(
                            psw[hb + 32 : hb + 64, :, :], pall[hb : hb + 32, :, :]
                        )
                    for jc in range(4):
                        tB = rt.tile([P, 512], BF16, tag="tB")
                        nc.gpsimd.tensor_mul(tB[:], psw[:, jc, :], sinb[:, sls])
                        nc.gpsimd.tensor_add(
                            dstT[:, jc, sls], dstT[:, jc, sls], tB[:]
                        )

                def proj_slice(sl, fillers=()):
                    # fillers: attention-block thunks emitted every 3rd
                    # projection group, so ScalarE exps / PE matmuls / DVE
                    # copies interleave at fine grain with no FIFO head-block
                    sls = ts(sl, 512)
                    if sl == 0:
                        xs = xs0
                    else:
                        xs = x1.tile([P, 8, 512], BF16, tag="xs", name="xs")
                        for kc in range(8):
                            nc.sync.dma_start(xs[:, kc, :], xT3[:, kc, sls])
                    qall = rt.tile([P, 4, 512], BF16, tag="pall", name="qall")
                    kall = rt.tile([P, 4, 512], BF16, tag="pall", name="kall")

                    def proj_group(w_s, pall, dstT, jc):
                        pq = psA.tile([P, 512], F32, tag="p1", name="pq")
                        for kc in range(8):
                            nc.tensor.matmul(
                                pq[:], w_s[:, kc, ts(jc, P)], xs[:, kc, :],
                                start=(kc == 0), stop=(kc == 7),
                            )
                        nc.vector.tensor_copy(pall[:, jc, :], pq[:])
                        nc.vector.tensor_mul(
                            dstT[:, jc, sls], pall[:, jc, :], cosb[:, sls]
                        )

                    def v_group(t4i):
                        pv = psA.tile([P, 512], F32, tag="p1", name="pv")
                        for kc in range(8):
                            nc.tensor.matmul(
                                pv[:], xs[:, kc, ts(t4i, P)], wv_s[:, kc, :],
                                start=(kc == 0), stop=(kc == 7),
                            )
                        nc.vector.tensor_copy(
                            vA[:, sl * 4 + t4i, :, 0:64],
                            pv.rearrange("p (h c) -> p h c", h=8),
                        )

                    groups = (
                        [lambda jc=jc: proj_group(wq_s, qall, qT, jc) for jc in range(4)]
                        + [lambda jc=jc: proj_group(wk_s, kall, kT, jc) for jc in range(4)]
                        + [lambda t=t: v_group(t) for t in range(4)]
                    )
                    fi = 0
                    for gi, g in enumerate(groups):
                        g()
                        if gi % 3 == 2 and fi < len(fillers):
                            fillers[fi]()
                            fi += 1
                    rope_finish(qall, qT, sls)
                    rope_finish(kall, kT, sls)
                    while fi < len(fillers):
                        fillers[fi]()
                        fi += 1

                def attention_block(j, hc):
                    pa0 = psC.tile([65, 512], F32, tag="pa", name="pa0")
                    pa1 = psC.tile([65, 512], F32, tag="pa", name="pa1")
                    last = 4 * j + 3
                    for i in range(last + 1):
                        m = i - 4 * j
                        w0 = max(m, 0) * P
                        sc = psB.tile([P, 1024], F32, tag="sc")
                        nc.tensor.matmul(
                            sc[:, w0:512], kT[0:64, hc, ts(i, P)],
                            qT[0:64, hc, j * 512 + w0 : (j + 1) * 512],
                            start=True, stop=True,
                        )
                        nc.tensor.matmul(
                            sc[:, 512 + w0 : 1024], kT[64:P, hc, ts(i, P)],
                            qT[64:P, hc, j * 512 + w0 : (j + 1) * 512],
                            start=True, stop=True,
                        )
                        pt = ptp.tile([P, 1024], BF16, tag="pt")
                        if m < 0:
                            nc.scalar.activation(pt[:], sc[:], Act.Exp)
                        else:
                            nc.scalar.activation(
                                pt[:, w0:512], sc[:, w0:512], Act.Exp
                            )
                            nc.scalar.activation(
                                pt[:, 512 + w0 : 1024],
                                sc[:, 512 + w0 : 1024], Act.Exp,
                            )
                            nc.vector.tensor_mul(
                                pt[:, w0 : w0 + P], pt[:, w0 : w0 + P], trib[:]
                            )
                            nc.vector.tensor_mul(
                                pt[:, 512 + w0 : 512 + w0 + P],
                                pt[:, 512 + w0 : 512 + w0 + P], trib[:],
                            )
                        nc.tensor.matmul(
                            pa0[:, w0:512], vA[:, i, 2 * hc, :], pt[:, w0:512],
                            start=(i == 0), stop=(i == last),
                        )
                        nc.tensor.matmul(
                            pa1[:, w0:512], vA[:, i, 2 * hc + 1, :],
                            pt[:, 512 + w0 : 1024],
                            start=(i == 0), stop=(i == last),
                        )
                    den_d = den_tiles[j]
                    for h01, pa in ((0, pa0), (1, pa1)):
                        hb = h01 * 64
                        nc.vector.tensor_copy(
                            outT[hb : hb + 64, hc, ts(j, 512)], pa[0:64, :]
                        )
                        # cheap pa release: denominator row to SBUF, then to
                        # the j-batched DRAM staging tile (reciprocal is a
                        # multi-pass DVE composite — run it once per j on all
                        # 8 rows, not per head)
                        r = 2 * hc + h01
                        srow = rcp.tile([1, 512], F32, tag="srow")
                        nc.vector.tensor_copy(srow[:], pa[64:65, :])
                        nc.sync.dma_start(den_d[r : r + 1, :], srow[:])

                def norm_block(j):
                    # one batched reciprocal over the j's 8 denominator rows,
                    # broadcast the rows back through DRAM, multiply outT's
                    # j-block in place
                    den_sb = rcp.tile([8, 512], F32, tag="densb")
                    nc.sync.dma_start(den_sb[:], den_tiles[j][:])
                    rec = rcp.tile([8, 512], BF16, tag="rec")
                    with nc.allow_low_precision(reason="bf16 softmax normalizer"):
                        nc.vector.reciprocal(rec[:], den_sb[:])
                    rec_d = drm.tile([8, 512], BF16, tag="recd", name="recd")
                    nc.sync.dma_start(rec_d[:], rec[:])
                    for hc in range(4):
                        for h01 in range(2):
                            r = 2 * hc + h01
                            hb = h01 * 64
                            rb = rbp.tile([P, 512], BF16, tag="rb")
                            nc.sync.dma_start(
                                rb[hb : hb + 64, :],
                                rec_d[r : r + 1, :].broadcast_to((64, 512)),
                            )
                            nc.vector.tensor_mul(
                                outT[hb : hb + 64, hc, ts(j, 512)],
                                outT[hb : hb + 64, hc, ts(j, 512)],
                                rb[hb : hb + 64, :],
                            )

                def p3_group(j, sts):
                    for st in sts:
                        for half in range(2):
                            py = psA.tile([P, 512], F32, tag="p1", name="py")
                            for jc in range(4):
                                nc.tensor.matmul(
                                    py[:], outT[:, jc, ts(st, P)],
                                    wo_s[:, jc, half * 512 : (half + 1) * 512],
                                    start=(jc == 0), stop=(jc == 3),
                                )
                            yo = ysb.tile([P, 512], F32, tag="yo")
                            nc.vector.tensor_copy(yo[:], py[:])
                            nc.sync.dma_start(
                                y_ap[ts(st, P), half * 512 : (half + 1) * 512],
                                yo[:],
                            )

                # attention(j) needs only key slices <= j, so its blocks fill
                # projection slice j+2: ScalarE chews block j's exps while PE
                # runs slice j+2's projections; blocks 2 and 3 then run with
                # the earlier blocks' output projections slotted into their
                # PE stream
                den_tiles = {}

                def att_fillers(j):
                    den_tiles[j] = drm.tile([8, 512], F32, tag="dend", name="dend")
                    fs = [lambda hc=hc: attention_block(j, hc) for hc in range(3)]
                    fs.append(lambda: (attention_block(j, 3), norm_block(j)))
                    return fs

                proj_slice(0)
                proj_slice(1)
                proj_slice(2, att_fillers(0))
                proj_slice(3, att_fillers(1))
                den_tiles[2] = drm.tile([8, 512], F32, tag="dend", name="dend")
                for hc in range(4):
                    attention_block(2, hc)
                    p3_group(0, [hc])
                norm_block(2)
                den_tiles[3] = drm.tile([8, 512], F32, tag="dend", name="dend")
                for hc in range(4):
                    attention_block(3, hc)
                    p3_group(1, [4 + hc])
                    if hc >= 1:
                        p3_group(2, [8 + hc - 1])
                norm_block(3)
                p3_group(2, [11])
                p3_group(3, [12, 13, 14, 15])

    nc.compile()
    return nc


def prep_core_inputs(x, token_ids, Wq, Wk, Wv, Wo, core):
    import ml_dtypes

    bf16 = ml_dtypes.bfloat16
    b, half = divmod(core, 2)
    rows = []
    for h in range(half * 8, half * 8 + 8):
        base = h * DH
        rows.extend(base + np.arange(0, DH, 2))
        rows.extend(base + np.arange(1, DH, 2))
    rows = np.asarray(rows)
    cols = np.arange(half * 512, half * 512 + 512)

    f32 = np.float32
    inv = THETA ** (-np.arange(0, DH, 2, dtype=np.float64) / DH)
    ang = np.asarray(token_ids, dtype=np.float64)[None, :] * inv[:, None]
    cosT = np.tile(np.cos(ang), (4, 1)).astype(bf16)
    # sign folded per DESTINATION row (the swap happens in the DMA, so the
    # mul is row-aligned): even-dim rows get -sin (r1 = x1 c - x2 s), odd-dim
    # rows get +sin (r2 = x2 c + x1 s)
    sin_block = np.concatenate([-np.sin(ang), np.sin(ang)], axis=0)
    sinT = np.tile(sin_block, (2, 1)).astype(bf16)
    tri = (np.arange(P)[:, None] <= np.arange(P)[None, :]).astype(bf16)
    return {
        "xT": np.ascontiguousarray(np.asarray(x, f32)[b].T).astype(bf16),
        "wqT": np.ascontiguousarray((np.asarray(Wq, f32)[rows] * 0.125).T).astype(bf16),
        "wkT": np.ascontiguousarray(np.asarray(Wk, f32)[rows].T).astype(bf16),
        "wvT": np.ascontiguousarray(np.asarray(Wv, f32)[cols].T).astype(bf16),
        "woT": np.ascontiguousarray(np.asarray(Wo, f32)[:, cols].T).astype(bf16),
        "cosT": cosT,
        "sinT": sinT,
        "tri": tri,
    }


def get_nc():
    if "nc" not in _CACHE:
        _CACHE["nc"] = build_nc()
    return _CACHE["nc"]


def run_cores(in_maps, trace=False):
    from concourse.bass_utils import run_bass_kernel_spmd

    return run_bass_kernel_spmd(
        get_nc(), in_maps, core_ids=list(range(N_CORES)), trace=trace
    )


def kernel(x, token_ids, Wq, Wk, Wv, Wo):
    in_maps = [
        prep_core_inputs(x, token_ids, Wq, Wk, Wv, Wo, c) for c in range(N_CORES)
    ]
    res = run_cores(in_maps)
    y = np.empty((B, S, D), np.float32)
    for b in range(B):
        y[b] = res.results[2 * b]["y"] + res.results[2 * b + 1]["y"]
    return y
